# revision 13
# baseline (speedup 1.0000x reference)
"""Trainium2 Bass kernel for nn_AdapterController (moe_routing).

Math (per sentence):
  z = LayerNorm(x) * g + b                      [S, D]
  probs = softmax(BN(mean_s z) @ Wr + br)       [E]
  idx = argmax(probs); gate = probs[idx]
  y = (relu(z @ W_down[idx] + b_down[idx]) @ W_up[idx] + b_up[idx]) * gate

Strategy: data-parallel over batch (8 sentences per core, no collectives).

V5 design (single input copy; one PE pass for stats AND router):
  - x ships ONCE, d-major, in 96-token chunks. Each chunk record is 129
    cols: [x (96) | folded router Wr hi/lo (16) | zeros (16) | ones (1)].
  - passA: per (chunk, dc) ONE matmul with stationary = record[0:128]
    (so FWL engages) and moving = record[0:129]. The PSUM block then
    holds: rows 0-95 = x_c^T x_c (diag -> per-token sum of squares),
    rows 96-111 = Wr^T x_c (router logits rows), col 128 = per-token
    sum (mean), accumulated over dc. Gram diag + mean + router in one
    streaming pass with zero extra stationary-load traffic.
  - LN is FUSED into mm1 (h = rs_t*(x @ Wd) - (mu_t*rs_t)*colsum(Wd)+bd),
    so the normalize pass over [S, D] never materializes.
  - Expert selection: data-dependent dma_gather of W_down (+bd/-colsum
    appended per row) and W_up rows; gate applied during PSUM eviction.
  - All PSUM tiles are full banks; evictions split ACT/DVE.
"""

import sys

if "/opt/trn_rl_repo" not in sys.path:
    sys.path.insert(0, "/opt/trn_rl_repo")

from contextlib import ExitStack

import ml_dtypes
import numpy as np

import concourse.bacc as bacc
import concourse.bass as bass
import concourse.bass_isa as bass_isa
import concourse.tile as tile
from concourse import mybir
from concourse.bass_utils import run_bass_kernel_spmd

B, S, D, H, E = 64, 1024, 1024, 64, 8
NCORES = 8
BLOC = B // NCORES
P = 128
DC = D // P          # 8 contraction chunks
M = 96               # tokens per chunk
W = 129              # chunk record: x(96) | wr(16) | pad(16) | ones(1)
C = 11               # chunks per sentence (1056 >= S; last has 64 real)
TC = S // P          # 128-token chunks for mm2/output
HP = H + 1           # h rows + ones row for the up-bias matmul
NWU = 80             # wu gather indices padded to a multiple of 16
NG = 208             # merged gather rows per expert: 128 wd + 80 wu
GROUPS = [(0, 3), (3, 3), (6, 3), (9, 2)]  # (first chunk, n) per psum bank
EPS = 1e-5
FP32 = mybir.dt.float32
BF16 = mybir.dt.bfloat16
I16 = mybir.dt.int16

_CACHE = {}

AL = mybir.AluOpType


def _build_kernel():
    nc = bacc.Bacc(
        "TRN2",
        target_bir_lowering=False,
        debug=False,
        enable_asserts=False,
        num_devices=NCORES,
    )
    xt_ext = nc.dram_tensor("xt", [BLOC, P, DC, C, W], BF16, kind="ExternalInput").ap()
    ucbc_ext = nc.dram_tensor("ucbc", [P, 2, E], FP32, kind="ExternalInput").ap()
    wdg_ext = nc.dram_tensor("wdg", [E * NG, D], BF16, kind="ExternalInput").ap()
    ident_ext = nc.dram_tensor("ident", [P, P], FP32, kind="ExternalInput").ap()
    maskg_ext = nc.dram_tensor("maskg", [M, 3 * M], BF16, kind="ExternalInput").ap()
    iota_ext = nc.dram_tensor("iota", [P, E], FP32, kind="ExternalInput").ap()
    wdb_ext = nc.dram_tensor("wdb", [P, NG // 16], I16, kind="ExternalInput").ap()
    out_ext = nc.dram_tensor("out", [BLOC, P, TC, D], BF16, kind="ExternalOutput").ap()

    with tile.TileContext(nc) as tc:
        _body(tc, out_ext, xt_ext, ucbc_ext, wdg_ext,
              ident_ext, maskg_ext, iota_ext, wdb_ext)

    nc.compile()
    return nc


def _body(tc, out_ext, xt_ext, ucbc_ext, wdg_ext,
          ident_ext, maskg_ext, iota_ext, wdb_ext):
    nc = tc.nc
    with ExitStack() as ctx:
        consts = ctx.enter_context(tc.tile_pool(name="consts", bufs=1))
        xt_pool = ctx.enter_context(tc.tile_pool(name="xtp", bufs=4))
        y_pool = ctx.enter_context(tc.tile_pool(name="yp", bufs=2))
        st_pool = ctx.enter_context(tc.tile_pool(name="stp", bufs=2))
        rb_pool = ctx.enter_context(tc.tile_pool(name="rbp", bufs=4))
        w_pool = ctx.enter_context(tc.tile_pool(name="wp", bufs=4))
        g_pool = ctx.enter_context(tc.tile_pool(name="gp", bufs=4))
        e_pool = ctx.enter_context(tc.tile_pool(name="ep", bufs=2))
        junk_pool = ctx.enter_context(tc.tile_pool(name="junk", bufs=1))
        pp_a = ctx.enter_context(tc.tile_pool(name="ppa", bufs=3, space="PSUM"))
        pp_1 = ctx.enter_context(tc.tile_pool(name="pp1", bufs=2, space="PSUM"))
        pp_y = ctx.enter_context(tc.tile_pool(name="ppy", bufs=2, space="PSUM"))
        pp_m = ctx.enter_context(tc.tile_pool(name="ppm", bufs=1, space="PSUM"))

        # constants
        ident_sb = consts.tile([P, P], FP32)
        nc.scalar.dma_start(ident_sb, ident_ext)
        ucbc = consts.tile([P, 2, E], FP32)
        nc.scalar.dma_start(ucbc, ucbc_ext)
        maskg = consts.tile([M, 3 * M], BF16)
        nc.scalar.dma_start(maskg, maskg_ext)
        iota_sb = consts.tile([P, E], FP32)
        nc.scalar.dma_start(iota_sb, iota_ext)
        wdb_sb = consts.tile([P, NG // 16], I16)
        nc.scalar.dma_start(wdb_sb, wdb_ext)

        junk_g = junk_pool.tile([P, 3 * M], FP32)   # gram tt scratch
        junk_xr = junk_pool.tile([2 * E, C * M], FP32)
        junk8 = junk_pool.tile([P, E], FP32)

        # hT double-buffer with a persistent ones row (row H)
        hT_all = junk_pool.tile([HP, 2, S], BF16)
        nc.vector.memset(hT_all[H:HP], 1.0)
        # sigma accumulator with rows 96:128 persistently zero
        sg_all = junk_pool.tile([P, 2, 1], FP32)
        nc.vector.memset(sg_all[M:P], 0.0)

        state = {}

        def front(b):
            # ---- load x (one descriptor per partition) ----
            xt = xt_pool.tile([P, DC, C, W], BF16, tag="xt")
            nc.sync.dma_start(xt, xt_ext[b])

            # ---- passA: gram diag + mean (ones col) + router rows;
            #      extraction per group: ssq (DVE), mean col + xr (ACT) ----
            ssq = st_pool.tile([P, C], FP32, tag="ssq")
            mur = st_pool.tile([P, C], FP32, tag="mur")
            xrT = st_pool.tile([2 * E, C, M], FP32, tag="xrT")
            for g, (c0, nch) in enumerate(GROUPS):
                ps = pp_a.tile([P, 512], FP32, tag="psA")
                for dc in range(DC):
                    for i in range(nch):
                        nc.tensor.matmul(
                            ps[:, i * W : (i + 1) * W],
                            xt[:, dc, c0 + i, 0:P], xt[:, dc, c0 + i],
                            start=(dc == 0 and i == 0),
                            stop=(dc == DC - 1 and i == nch - 1),
                        )
                pv = ps[:, 0 : nch * W].rearrange("p (c w) -> p c w", w=W)
                nc.vector.tensor_tensor(
                    junk_g[0:M, 0 : nch * M].rearrange("p (c m) -> p c m", m=M),
                    pv[0:M, 0:nch, 0:M],
                    maskg[:, 0 : nch * M].rearrange("p (c m) -> p c m", m=M),
                    AL.mult,
                )
                nc.vector.tensor_reduce(
                    ssq[0:M, c0 : c0 + nch],
                    junk_g[0:M, 0 : nch * M].rearrange("p (c m) -> p c m", m=M),
                    axis=mybir.AxisListType.X, op=AL.add,
                )
                nc.scalar.copy(mur[0:M, c0 : c0 + nch], pv[0:M, 0:nch, P])
                nc.scalar.copy(
                    xrT[:, c0 : c0 + nch],
                    pv[M : M + 2 * E, 0:nch, 0:M],
                )

            # ---- rs = D/sqrt(ssq*D - mur^2 + eps*D^2); mrs = mur*rs/D ----
            sg = sg_all[:, b % 2]
            rsmrs = st_pool.tile([P, 2 * C], FP32, tag="rsmrs")
            rs = rsmrs[0:M, 0:C]
            mrs = rsmrs[0:M, C : 2 * C]
            m2 = st_pool.tile([P, C], FP32, tag="m2")
            nc.vector.tensor_tensor(m2[0:M], mur[0:M], mur[0:M], AL.mult)
            v1 = st_pool.tile([P, C], FP32, tag="v1")
            nc.vector.scalar_tensor_tensor(
                v1[0:M], ssq[0:M], float(D), m2[0:M], AL.mult, AL.subtract
            )
            nc.vector.tensor_scalar_add(v1[0:M], v1[0:M], float(EPS * D * D))
            rcp = st_pool.tile([P, C], FP32, tag="rcp")
            nc.vector.reciprocal(rcp[0:M], v1[0:M])
            nc.scalar.activation(
                rs, rcp[0:M], mybir.ActivationFunctionType.Sqrt,
                scale=float(D) * float(D),
            )
            nc.vector.scalar_tensor_tensor(
                mrs, mur[0:M], 1.0 / float(D), rs, AL.mult, AL.mult,
                accum_out=sg[0:M],
            )
            sgr = st_pool.tile([P, 1], FP32, tag="sgr")
            nc.gpsimd.partition_all_reduce(sgr, sg, P, bass_isa.ReduceOp.add)

            # ---- transpose rs/mrs to rows, broadcast to partitions 0-63 ----
            ps_t = pp_m.tile([2 * C, P + 2 * E], FP32, tag="pst")
            nc.tensor.transpose(ps_t[:, 0:M], rsmrs[0:M], ident_sb[0:M, 0:M])
            rmT = st_pool.tile([2 * C, M], BF16, tag="rmT")
            nc.scalar.copy(rmT, ps_t[:, 0:M])
            rrow = st_pool.tile([1, 2, C, W], BF16, tag="rrow")
            nc.sync.dma_start(rrow[:, 0, :, 0:M], rmT[0:C])
            nc.sync.dma_start(rrow[:, 1, :, 0:M], rmT[C : 2 * C])
            rb = rb_pool.tile([H, 2, C, W], BF16, tag="rb")
            nc.gpsimd.partition_broadcast(rb, rrow)

            # ---- router logits: lraw[e] = sum_t xr[e,t]*rs_t ----
            lraw = st_pool.tile([2 * E, 1], FP32, tag="lraw")
            nc.vector.scalar_tensor_tensor(
                junk_xr.rearrange("p (c m) -> p c m", m=M),
                xrT, 1.0, rb[0 : 2 * E, 0, :, 0:M],
                AL.mult, AL.mult,
                accum_out=lraw,
            )
            ps_lt = ps_t[0:1, P : P + 2 * E]
            nc.tensor.transpose(ps_lt, lraw, ident_sb[0 : 2 * E, 0 : 2 * E])
            lrawT2 = st_pool.tile([1, 2 * E], FP32, tag="lrawT2")
            nc.scalar.copy(lrawT2, ps_lt)
            lrawT = st_pool.tile([1, E], FP32, tag="lrawT")
            nc.vector.tensor_tensor(
                lrawT, lrawT2[:, 0:E], lrawT2[:, E : 2 * E], AL.add
            )
            # logits = lrawT - sigma*u + c   (ucbc[:,0] = -u, ucbc[:,1] = c)
            logits = st_pool.tile([1, E], FP32, tag="logits")
            nc.vector.scalar_tensor_tensor(
                logits, ucbc[0:1, 0], sgr[0:1], lrawT, AL.mult, AL.add
            )
            nc.vector.tensor_tensor(logits, logits, ucbc[0:1, 1], AL.add)

            # ---- gating (single partition, then broadcast) ----
            lmax = st_pool.tile([1, 1], FP32, tag="lmax")
            nc.vector.tensor_reduce(lmax, logits, axis=mybir.AxisListType.X, op=AL.max)
            u0 = st_pool.tile([1, E], FP32, tag="u0")
            nc.vector.tensor_scalar(u0, logits, lmax, None, AL.subtract)
            ex = st_pool.tile([1, E], FP32, tag="ex")
            nc.vector.tensor_scalar(ex, u0, 0.25, 1.0, AL.mult, AL.add)
            for coef in (3.0, 2.0, 1.0):
                nc.vector.tensor_mul(ex, ex, u0)
                nc.vector.tensor_scalar(ex, ex, 1.0 / coef, 1.0, AL.mult, AL.add)
            denom = st_pool.tile([1, 1], FP32, tag="denom")
            nc.vector.tensor_reduce(denom, ex, axis=mybir.AxisListType.X, op=AL.add)
            oh = st_pool.tile([1, E], FP32, tag="oh")
            nc.vector.tensor_scalar(oh, logits, lmax, None, AL.is_equal)
            # gidx = [gate, idx*128, idx*80] on one partition, then broadcast
            idxf = st_pool.tile([1, 1], FP32, tag="idxf")
            nc.vector.scalar_tensor_tensor(
                junk8[0:1], oh, 1.0, iota_sb[0:1], AL.mult, AL.mult,
                accum_out=idxf,
            )
            gidx = st_pool.tile([1, 3], FP32, tag="gidx")
            nc.vector.reciprocal(gidx[:, 0:1], denom)
            nc.vector.tensor_scalar_mul(gidx[:, 1:2], idxf, float(NG))
            gidx_b = g_pool.tile([P, 3], FP32, tag="gidxb")
            nc.gpsimd.partition_broadcast(gidx_b, gidx)
            gate = gidx_b[:, 0:1]
            w_idxs = st_pool.tile([P, NG // 16], I16, tag="widx")
            nc.vector.tensor_scalar_add(w_idxs, wdb_sb, gidx_b[:, 1:2])
            wg = w_pool.tile([P, 2, D], BF16, tag="wg")
            nc.gpsimd.dma_gather(wg, wdg_ext, w_idxs, NG, NG, D)

            state[b] = (xt, rb, wg, gate)

        def back(b):
            xt, rb, wg, gate = state.pop(b)
            wu_eff = wg[0:HP, 1]            # [HP, D]
            bd = wg[0:H, 0, 512:513]        # [H, 1] b_down (folded)
            ncs = wg[0:H, 0, 513:514]       # [H, 1] -colsum(Wd)
            hT = hT_all[:, b % 2]           # [HP, S]; row H is ones

            # ---- mm1 + fused-LN affine + relu -> hT [HP, S] bf16 ----
            for g, (c0, nch) in enumerate(GROUPS):
                ps = pp_1.tile([H, 512], FP32, tag="ps1")
                for dc in range(DC):
                    nc.tensor.matmul(
                        ps[:, 0 : nch * W],
                        wg[:, 0, dc * H : (dc + 1) * H],
                        xt[:, dc, c0 : c0 + nch],
                        start=(dc == 0), stop=(dc == DC - 1),
                    )
                pv = ps[:, 0 : nch * W].rearrange("p (c w) -> p c w", w=W)
                nreal = nch * M if g < 3 else M + H  # last group: 96+64 real
                e1 = e_pool.tile([H, 3 * M], FP32, tag="e1")
                e1v = e1[:, 0 : nch * M].rearrange("p (c m) -> p c m", m=M)
                nc.vector.tensor_tensor(
                    e1v, pv[:, 0:nch, 0:M],
                    rb[0:H, 0, c0 : c0 + nch, 0:M], AL.mult,
                )
                e2 = e_pool.tile([H, 3 * M], FP32, tag="e2")
                nc.vector.scalar_tensor_tensor(
                    e2[:, 0 : nch * M].rearrange("p (c m) -> p c m", m=M),
                    rb[0:H, 1, c0 : c0 + nch, 0:M], ncs, e1v,
                    AL.mult, AL.add,
                )
                nc.scalar.activation(
                    hT[0:H, c0 * M : c0 * M + nreal], e2[:, 0:nreal],
                    mybir.ActivationFunctionType.Relu, bias=bd,
                )

            # ---- mm2; gate applied in the PSUM->SBUF eviction ----
            y_sb = y_pool.tile([P, TC, D], BF16, tag="y")
            for t in range(TC):
                for half in range(2):
                    sl = slice(half * 512, (half + 1) * 512)
                    ps_y = pp_y.tile([P, 512], FP32)
                    nc.tensor.matmul(
                        ps_y,
                        hT[:, t * P : (t + 1) * P],
                        wu_eff[:, sl],
                        start=True,
                        stop=True,
                    )
                    if (t * 2 + half) % 8 < 5:
                        nc.scalar.mul(y_sb[:, t, sl], ps_y, gate)
                    else:
                        nc.vector.tensor_scalar_mul(y_sb[:, t, sl], ps_y, gate)
            nc.sync.dma_start(out_ext[b], y_sb)

        for b in range(4):
            front(b)
        for b in range(BLOC):
            back(b)
            if b + 4 < BLOC:
                front(b + 4)


def _fold_weights(inputs):
    g = np.asarray(inputs["ln_g"], np.float32)
    bb = np.asarray(inputs["ln_b"], np.float32)
    bn_g = np.asarray(inputs["bn_g"], np.float32)
    bn_b = np.asarray(inputs["bn_b"], np.float32)
    bn_mean = np.asarray(inputs["bn_mean"], np.float32)
    bn_var = np.asarray(inputs["bn_var"], np.float32)
    Wr = np.asarray(inputs["Wr"], np.float32)
    br = np.asarray(inputs["br"], np.float32)
    W_down = np.asarray(inputs["W_down"], np.float32)
    b_down = np.asarray(inputs["b_down"], np.float32)
    W_up = np.asarray(inputs["W_up"], np.float32)
    b_up = np.asarray(inputs["b_up"], np.float32)

    q = 1.0 / np.sqrt(bn_var + np.float32(EPS))
    wr_f = ((g * q * bn_g / np.float32(S))[:, None] * Wr).astype(np.float32)
    c = (((bb - bn_mean) * q * bn_g + bn_b) @ Wr + br).astype(np.float32)
    u = wr_f.sum(axis=0)  # [E]
    ucbc = np.stack([-u, c], axis=0)  # [2, E]
    ucbc = np.ascontiguousarray(
        np.broadcast_to(ucbc[None], (P, 2, E)).astype(np.float32)
    )

    wd_f = (g[None, :, None] * W_down).astype(ml_dtypes.bfloat16)  # [E, D, H]
    cs = wd_f.astype(np.float32).sum(axis=1)  # [E, H] colsums of bf16 weights
    bd_f = (b_down + np.einsum("d,edh->eh", bb, W_down)).astype(np.float32)
    wu_f = np.concatenate([W_up, b_up[:, None, :]], axis=1).astype(
        ml_dtypes.bfloat16
    )  # [E, HP, D]

    # merged gather table: rows 0-127 of each expert pair xt partition p
    # with wd cols for all dc (cols 512/513 carry bd / -colsum for
    # h = p % 64); rows 128-207 are the W_up/b_up rows.
    wdg = np.zeros((E, NG, D), dtype=ml_dtypes.bfloat16)
    wdg[:, :P, : DC * H] = (
        wd_f.reshape(E, DC, P, H).transpose(0, 2, 1, 3).reshape(E, P, DC * H)
    )
    bdcs = np.stack([bd_f, -cs], axis=-1)  # [E, H, 2]
    wdg[:, :P, 512:514] = np.concatenate([bdcs, bdcs], axis=1).astype(
        ml_dtypes.bfloat16
    )
    wdg[:, P : P + HP] = wu_f
    wdg = wdg.reshape(E * NG, D)

    # gather ucode reads index i from slot [16 + i%16, i//16]; mirror into
    # partitions 0..15 too for the simulator's interpretation
    def _idx_table(n):
        t = np.zeros((P, n // 16), dtype=np.int16)
        t[:16, :] = np.arange(n, dtype=np.int16).reshape(n // 16, 16).T
        t[16:32, :] = t[:16, :]
        return t

    wdb = _idx_table(NG)
    iota = np.ascontiguousarray(
        np.broadcast_to(np.arange(E, dtype=np.float32)[None], (P, E))
    )
    maskg = np.ascontiguousarray(
        np.tile(np.eye(M, dtype=ml_dtypes.bfloat16), (1, 3))
    )

    wr_hi = wr_f.astype(ml_dtypes.bfloat16)
    wr_lo = (wr_f - wr_hi.astype(np.float32)).astype(ml_dtypes.bfloat16)
    wr2 = np.concatenate([wr_hi, wr_lo], axis=1)  # [D, 2E]

    return {
        "ucbc": ucbc,
        "wdg": np.ascontiguousarray(wdg),
        "ident": np.eye(P, dtype=np.float32),
        "maskg": maskg,
        "iota": iota,
        "wdb": np.ascontiguousarray(wdb),
    }, wr2


def make_in_maps(inputs):
    params, wr2 = _fold_weights(inputs)
    x = np.asarray(inputs["x"], np.float32)
    x_bf = x.astype(ml_dtypes.bfloat16)
    wr_dc = np.ascontiguousarray(wr2.reshape(DC, P, 2 * E))  # [DC, P, 16]
    in_maps = []
    for i in range(NCORES):
        m = dict(params)
        xb = x_bf[i * BLOC : (i + 1) * BLOC]  # [BLOC, S, D]
        xp = np.zeros((BLOC, C * M, D), dtype=ml_dtypes.bfloat16)
        xp[:, :S] = xb
        xt = np.zeros((BLOC, P, DC, C, W), dtype=ml_dtypes.bfloat16)
        # [b, c, j, dc, p] -> [b, p, dc, c, j]
        xt[..., 0:M] = xp.reshape(BLOC, C, M, DC, P).transpose(0, 4, 3, 1, 2)
        xt[..., M : M + 2 * E] = wr_dc.transpose(1, 0, 2)[None, :, :, None, :]
        xt[..., P] = 1.0
        m["xt"] = np.ascontiguousarray(xt)
        in_maps.append(m)
    return in_maps


def get_nc():
    if "nc" not in _CACHE:
        _CACHE["nc"] = _build_kernel()
    return _CACHE["nc"]


def kernel(**inputs) -> np.ndarray:
    nc = get_nc()
    in_maps = make_in_maps(inputs)
    res = run_bass_kernel_spmd(nc, in_maps, core_ids=list(range(NCORES)))
    _CACHE["last_result"] = res
    out = np.concatenate(
        [
            np.asarray(res.results[i]["out"])
            .astype(np.float32)
            .transpose(0, 2, 1, 3)
            .reshape(BLOC, S, D)
            for i in range(NCORES)
        ],
        axis=0,
    )
    return out


if __name__ == "__main__":
    nc = get_nc()
    print("build + compile OK")


# revision 14
# speedup vs baseline: 1.2065x; 1.2065x over previous
"""Trainium2 Bass kernel for nn_AdapterController (moe_routing).

Math (per sentence):
  z = LayerNorm(x) * g + b                      [S, D]
  probs = softmax(BN(mean_s z) @ Wr + br)       [E]
  idx = argmax(probs); gate = probs[idx]
  y = (relu(z @ W_down[idx] + b_down[idx]) @ W_up[idx] + b_up[idx]) * gate

Strategy: data-parallel over batch (8 sentences per core, no collectives).

V5 design (single input copy; one PE pass for stats AND router):
  - x ships ONCE, d-major, in 96-token chunks. Each chunk record is 129
    cols: [x (96) | folded router Wr hi/lo (16) | zeros (16) | ones (1)].
  - passA: per (chunk, dc) ONE matmul with stationary = record[0:128]
    (so FWL engages) and moving = record[0:129]. The PSUM block then
    holds: rows 0-95 = x_c^T x_c (diag -> per-token sum of squares),
    rows 96-111 = Wr^T x_c (router logits rows), col 128 = per-token
    sum (mean), accumulated over dc. Gram diag + mean + router in one
    streaming pass with zero extra stationary-load traffic.
  - LN is FUSED into mm1 (h = rs_t*(x @ Wd) - (mu_t*rs_t)*colsum(Wd)+bd),
    so the normalize pass over [S, D] never materializes.
  - Expert selection: data-dependent dma_gather of W_down (+bd/-colsum
    appended per row) and W_up rows; gate applied during PSUM eviction.
  - All PSUM tiles are full banks; evictions split ACT/DVE.
"""

import sys

if "/opt/trn_rl_repo" not in sys.path:
    sys.path.insert(0, "/opt/trn_rl_repo")

from contextlib import ExitStack

import ml_dtypes
import numpy as np

import concourse.bacc as bacc
import concourse.bass as bass
import concourse.bass_isa as bass_isa
import concourse.tile as tile
from concourse import mybir
from concourse.ap import AP
from concourse.bass_utils import run_bass_kernel_spmd

B, S, D, H, E = 64, 1024, 1024, 64, 8
NCORES = 8
BLOC = B // NCORES
P = 128
DC = D // P          # 8 contraction chunks
M = 96               # tokens per chunk
W = 129              # chunk record: x(96) | wr(16) | pad(16) | ones(1)
C = 11               # chunks per sentence (1056 >= S; last has 64 real)
TC = S // P          # 128-token chunks for mm2/output
HP = H + 1           # h rows + ones row for the up-bias matmul
NWU = 80             # wu gather indices padded to a multiple of 16
FW = 1664            # per-partition expert record: wd(640) | wu(1024)
GROUPS = [(0, 3), (3, 3), (6, 3), (9, 2)]  # (first chunk, n) per psum bank
EPS = 1e-5
FP32 = mybir.dt.float32
BF16 = mybir.dt.bfloat16
I16 = mybir.dt.int16

_CACHE = {}

AL = mybir.AluOpType


def _build_kernel():
    nc = bacc.Bacc(
        "TRN2",
        target_bir_lowering=False,
        debug=False,
        enable_asserts=False,
        num_devices=NCORES,
    )
    xt_ext = nc.dram_tensor("xt", [BLOC, P, DC, C, W], BF16, kind="ExternalInput").ap()
    ucbc_ext = nc.dram_tensor("ucbc", [P, 2, E], FP32, kind="ExternalInput").ap()
    wdg_ext = nc.dram_tensor("wdg", [E * P, FW], BF16, kind="ExternalInput").ap()
    ident_ext = nc.dram_tensor("ident", [P, P], FP32, kind="ExternalInput").ap()
    maskg_ext = nc.dram_tensor("maskg", [M, 3 * M], BF16, kind="ExternalInput").ap()
    iota_ext = nc.dram_tensor("iota", [P, E], FP32, kind="ExternalInput").ap()
    out_ext = nc.dram_tensor("out", [BLOC, P, TC, D], BF16, kind="ExternalOutput").ap()

    with tile.TileContext(nc) as tc:
        _body(tc, out_ext, xt_ext, ucbc_ext, wdg_ext,
              ident_ext, maskg_ext, iota_ext)

    nc.compile()
    return nc


def _body(tc, out_ext, xt_ext, ucbc_ext, wdg_ext,
          ident_ext, maskg_ext, iota_ext):
    nc = tc.nc
    with ExitStack() as ctx:
        consts = ctx.enter_context(tc.tile_pool(name="consts", bufs=1))
        xt_pool = ctx.enter_context(tc.tile_pool(name="xtp", bufs=4))
        y_pool = ctx.enter_context(tc.tile_pool(name="yp", bufs=2))
        st_pool = ctx.enter_context(tc.tile_pool(name="stp", bufs=2))
        rb_pool = ctx.enter_context(tc.tile_pool(name="rbp", bufs=4))
        w_pool = ctx.enter_context(tc.tile_pool(name="wp", bufs=4))
        g_pool = ctx.enter_context(tc.tile_pool(name="gp", bufs=4))
        e_pool = ctx.enter_context(tc.tile_pool(name="ep", bufs=2))
        junk_pool = ctx.enter_context(tc.tile_pool(name="junk", bufs=1))
        pp_a = ctx.enter_context(tc.tile_pool(name="ppa", bufs=3, space="PSUM"))
        pp_1 = ctx.enter_context(tc.tile_pool(name="pp1", bufs=2, space="PSUM"))
        pp_y = ctx.enter_context(tc.tile_pool(name="ppy", bufs=2, space="PSUM"))
        pp_m = ctx.enter_context(tc.tile_pool(name="ppm", bufs=1, space="PSUM"))

        # constants
        ident_sb = consts.tile([P, P], FP32)
        nc.scalar.dma_start(ident_sb, ident_ext)
        ucbc = consts.tile([P, 2, E], FP32)
        nc.scalar.dma_start(ucbc, ucbc_ext)
        maskg = consts.tile([M, 3 * M], BF16)
        nc.scalar.dma_start(maskg, maskg_ext)
        iota_sb = consts.tile([P, E], FP32)
        nc.scalar.dma_start(iota_sb, iota_ext)

        junk_g = junk_pool.tile([P, 3 * M], FP32)   # gram tt scratch
        junk_xr = junk_pool.tile([2 * E, C * M], FP32)
        junk8 = junk_pool.tile([P, E], FP32)

        # hT double-buffer with a persistent ones row (row H)
        hT_all = junk_pool.tile([HP, 2, S], BF16)
        nc.vector.memset(hT_all[H:HP], 1.0)

        r_off = ctx.enter_context(nc.sync.register("goff"))

        state = {}

        def front(b):
            # ---- load x (one descriptor per partition) ----
            xt = xt_pool.tile([P, DC, C, W], BF16, tag="xt")
            nc.sync.dma_start(xt, xt_ext[b])

            # ---- passA: gram diag + mean (ones col) + router rows;
            #      extraction per group: ssq (DVE), mean col + xr (ACT) ----
            ssq = st_pool.tile([P, C], FP32, tag="ssq")
            mur = st_pool.tile([P, C], FP32, tag="mur")
            xrT = st_pool.tile([2 * E, C, M], FP32, tag="xrT")
            for g, (c0, nch) in enumerate(GROUPS):
                ps = pp_a.tile([P, 512], FP32, tag="psA")
                for dc in range(DC):
                    for i in range(nch):
                        nc.tensor.matmul(
                            ps[:, i * W : (i + 1) * W],
                            xt[:, dc, c0 + i, 0:P], xt[:, dc, c0 + i],
                            start=(dc == 0 and i == 0),
                            stop=(dc == DC - 1 and i == nch - 1),
                        )
                pv = ps[:, 0 : nch * W].rearrange("p (c w) -> p c w", w=W)
                nc.vector.tensor_tensor(
                    junk_g[0:M, 0 : nch * M].rearrange("p (c m) -> p c m", m=M),
                    pv[0:M, 0:nch, 0:M],
                    maskg[:, 0 : nch * M].rearrange("p (c m) -> p c m", m=M),
                    AL.mult,
                )
                nc.vector.tensor_reduce(
                    ssq[0:M, c0 : c0 + nch],
                    junk_g[0:M, 0 : nch * M].rearrange("p (c m) -> p c m", m=M),
                    axis=mybir.AxisListType.X, op=AL.add,
                )
                nc.scalar.copy(mur[0:M, c0 : c0 + nch], pv[0:M, 0:nch, P])
                nc.scalar.copy(
                    xrT[:, c0 : c0 + nch],
                    pv[M : M + 2 * E, 0:nch, 0:M],
                )

            # ---- rs = D/sqrt(ssq*D - mur^2 + eps*D^2); mrs = mur*rs/D ----
            rsmrs = st_pool.tile([P, 2 * C], FP32, tag="rsmrs")
            rs = rsmrs[0:M, 0:C]
            mrs = rsmrs[0:M, C : 2 * C]
            m2 = st_pool.tile([P, C], FP32, tag="m2")
            nc.vector.tensor_tensor(m2[0:M], mur[0:M], mur[0:M], AL.mult)
            v1 = st_pool.tile([P, C], FP32, tag="v1")
            nc.vector.scalar_tensor_tensor(
                v1[0:M], ssq[0:M], float(D), m2[0:M], AL.mult, AL.subtract
            )
            nc.vector.tensor_scalar_add(v1[0:M], v1[0:M], float(EPS * D * D))
            rcp = st_pool.tile([P, C], FP32, tag="rcp")
            nc.vector.reciprocal(rcp[0:M], v1[0:M])
            nc.scalar.activation(
                rs, rcp[0:M], mybir.ActivationFunctionType.Sqrt,
                scale=float(D) * float(D),
            )
            nc.vector.scalar_tensor_tensor(
                mrs, mur[0:M], 1.0 / float(D), rs, AL.mult, AL.mult,
            )

            # ---- transpose rs/mrs to rows, broadcast to partitions 0-63 ----
            ps_t = pp_m.tile([2 * C, P + 2 * E], FP32, tag="pst")
            nc.tensor.transpose(ps_t[:, 0:M], rsmrs[0:M], ident_sb[0:M, 0:M])
            rmT = st_pool.tile([2 * C, M], BF16, tag="rmT")
            nc.scalar.copy(rmT, ps_t[:, 0:M])
            rrow = st_pool.tile([1, 2, C, W], BF16, tag="rrow")
            nc.scalar.dma_start(rrow[:, :, :, 0:M], rmT[0 : 2 * C])
            # sigma = sum_t mu_t * rs_t  (pad-token mrs are zero)
            sig = st_pool.tile([1, 1], FP32, tag="sig")
            nc.vector.tensor_reduce(
                sig, rrow[:, 1, :, 0:M], axis=mybir.AxisListType.XY, op=AL.add
            )
            rb_r = st_pool.tile([2 * E, C, W], BF16, tag="rbr")
            nc.gpsimd.partition_broadcast(rb_r, rrow[:, 0])

            # ---- router logits: lraw[e] = sum_t xr[e,t]*rs_t ----
            lraw = st_pool.tile([2 * E, 1], FP32, tag="lraw")
            nc.vector.scalar_tensor_tensor(
                junk_xr.rearrange("p (c m) -> p c m", m=M),
                xrT, 1.0, rb_r[:, :, 0:M],
                AL.mult, AL.mult,
                accum_out=lraw,
            )
            ps_lt = ps_t[0:1, P : P + 2 * E]
            nc.tensor.transpose(ps_lt, lraw, ident_sb[0 : 2 * E, 0 : 2 * E])
            lrawT2 = st_pool.tile([1, 2 * E], FP32, tag="lrawT2")
            nc.scalar.copy(lrawT2, ps_lt)
            lrawT = st_pool.tile([1, E], FP32, tag="lrawT")
            nc.vector.tensor_tensor(
                lrawT, lrawT2[:, 0:E], lrawT2[:, E : 2 * E], AL.add
            )
            # logits = lrawT - sigma*u + c   (ucbc[:,0] = -u, ucbc[:,1] = c)
            logits = st_pool.tile([1, E], FP32, tag="logits")
            nc.vector.scalar_tensor_tensor(
                logits, ucbc[0:1, 0], sig, lrawT, AL.mult, AL.add
            )
            nc.vector.tensor_tensor(logits, logits, ucbc[0:1, 1], AL.add)

            # ---- gating (single partition, then broadcast) ----
            lmax = st_pool.tile([1, 1], FP32, tag="lmax")
            nc.vector.tensor_reduce(lmax, logits, axis=mybir.AxisListType.X, op=AL.max)
            u0 = st_pool.tile([1, E], FP32, tag="u0")
            nc.vector.tensor_scalar(u0, logits, lmax, None, AL.subtract)
            ex = st_pool.tile([1, E], FP32, tag="ex")
            nc.vector.tensor_scalar(ex, u0, 0.25, 1.0, AL.mult, AL.add)
            for coef in (3.0, 2.0, 1.0):
                nc.vector.tensor_mul(ex, ex, u0)
                nc.vector.tensor_scalar(ex, ex, 1.0 / coef, 1.0, AL.mult, AL.add)
            denom = st_pool.tile([1, 1], FP32, tag="denom")
            nc.vector.tensor_reduce(denom, ex, axis=mybir.AxisListType.X, op=AL.add)
            oh = st_pool.tile([1, E], FP32, tag="oh")
            nc.vector.tensor_scalar(oh, logits, lmax, None, AL.is_equal)
            # gidx = [gate, idx*128, idx*80] on one partition, then broadcast
            idxf = st_pool.tile([1, 1], FP32, tag="idxf")
            nc.vector.scalar_tensor_tensor(
                junk8[0:1], oh, 1.0, iota_sb[0:1], AL.mult, AL.mult,
                accum_out=idxf,
            )
            # expert weights via dynamic-offset DMA (contiguous block)
            off_i = st_pool.tile([1, 1], mybir.dt.int32, tag="offi")
            nc.vector.tensor_scalar_mul(off_i, idxf, float(P * FW))
            wg = w_pool.tile([P, FW], BF16, tag="wg")
            nc.sync.load(r_off, off_i[0:1, 0:1])
            nc.sync.dma_start(
                wg,
                AP(wdg_ext.tensor, offset=r_off, ap=[[FW, P], [1, FW]],
                   dep_tracking_offset=0),
            )
            gidx = st_pool.tile([1, 1], FP32, tag="gidx")
            nc.vector.reciprocal(gidx, denom)
            gidx_b = g_pool.tile([P, 1], FP32, tag="gidxb")
            nc.gpsimd.partition_broadcast(gidx_b, gidx)
            gate = gidx_b[:, 0:1]
            rb = rb_pool.tile([H, 2, C, W], BF16, tag="rb")
            nc.gpsimd.partition_broadcast(rb, rrow)

            state[b] = (xt, rb, wg, gate)

        def back(b):
            xt, rb, wg, gate = state.pop(b)
            wu_eff = wg[0:HP, 640:640 + D]  # [HP, D]
            bd = wg[0:H, 512:513]           # [H, 1] b_down (folded)
            ncs = wg[0:H, 513:514]          # [H, 1] -colsum(Wd)
            hT = hT_all[:, b % 2]           # [HP, S]; row H is ones

            # ---- mm1 + fused-LN affine + relu -> hT [HP, S] bf16 ----
            for g, (c0, nch) in enumerate(GROUPS):
                ps = pp_1.tile([H, 512], FP32, tag="ps1")
                for dc in range(DC):
                    nc.tensor.matmul(
                        ps[:, 0 : nch * W],
                        wg[:, dc * H : (dc + 1) * H],
                        xt[:, dc, c0 : c0 + nch],
                        start=(dc == 0), stop=(dc == DC - 1),
                    )
                pv = ps[:, 0 : nch * W].rearrange("p (c w) -> p c w", w=W)
                nreal = nch * M if g < 3 else M + H  # last group: 96+64 real
                e1 = e_pool.tile([H, 3 * M], FP32, tag="e1")
                e1v = e1[:, 0 : nch * M].rearrange("p (c m) -> p c m", m=M)
                nc.vector.tensor_tensor(
                    e1v, pv[:, 0:nch, 0:M],
                    rb[0:H, 0, c0 : c0 + nch, 0:M], AL.mult,
                )
                e2 = e_pool.tile([H, 3 * M], FP32, tag="e2")
                nc.vector.scalar_tensor_tensor(
                    e2[:, 0 : nch * M].rearrange("p (c m) -> p c m", m=M),
                    rb[0:H, 1, c0 : c0 + nch, 0:M], ncs, e1v,
                    AL.mult, AL.add,
                )
                nc.scalar.activation(
                    hT[0:H, c0 * M : c0 * M + nreal], e2[:, 0:nreal],
                    mybir.ActivationFunctionType.Relu, bias=bd,
                )

            # ---- mm2; gate applied in the PSUM->SBUF eviction ----
            y_sb = y_pool.tile([P, TC, D], BF16, tag="y")
            for t in range(TC):
                for half in range(2):
                    sl = slice(half * 512, (half + 1) * 512)
                    ps_y = pp_y.tile([P, 512], FP32)
                    nc.tensor.matmul(
                        ps_y,
                        hT[:, t * P : (t + 1) * P],
                        wu_eff[:, sl],
                        start=True,
                        stop=True,
                    )
                    if (t * 2 + half) % 8 < 5:
                        nc.scalar.mul(y_sb[:, t, sl], ps_y, gate)
                    else:
                        nc.vector.tensor_scalar_mul(y_sb[:, t, sl], ps_y, gate)
            nc.sync.dma_start(out_ext[b], y_sb)

        for b in range(4):
            front(b)
        for b in range(BLOC):
            back(b)
            if b + 4 < BLOC:
                front(b + 4)


def _fold_weights(inputs):
    g = np.asarray(inputs["ln_g"], np.float32)
    bb = np.asarray(inputs["ln_b"], np.float32)
    bn_g = np.asarray(inputs["bn_g"], np.float32)
    bn_b = np.asarray(inputs["bn_b"], np.float32)
    bn_mean = np.asarray(inputs["bn_mean"], np.float32)
    bn_var = np.asarray(inputs["bn_var"], np.float32)
    Wr = np.asarray(inputs["Wr"], np.float32)
    br = np.asarray(inputs["br"], np.float32)
    W_down = np.asarray(inputs["W_down"], np.float32)
    b_down = np.asarray(inputs["b_down"], np.float32)
    W_up = np.asarray(inputs["W_up"], np.float32)
    b_up = np.asarray(inputs["b_up"], np.float32)

    q = 1.0 / np.sqrt(bn_var + np.float32(EPS))
    wr_f = ((g * q * bn_g / np.float32(S))[:, None] * Wr).astype(np.float32)
    c = (((bb - bn_mean) * q * bn_g + bn_b) @ Wr + br).astype(np.float32)
    u = wr_f.sum(axis=0)  # [E]
    ucbc = np.stack([-u, c], axis=0)  # [2, E]
    ucbc = np.ascontiguousarray(
        np.broadcast_to(ucbc[None], (P, 2, E)).astype(np.float32)
    )

    wd_f = (g[None, :, None] * W_down).astype(ml_dtypes.bfloat16)  # [E, D, H]
    cs = wd_f.astype(np.float32).sum(axis=1)  # [E, H] colsums of bf16 weights
    bd_f = (b_down + np.einsum("d,edh->eh", bb, W_down)).astype(np.float32)
    wu_f = np.concatenate([W_up, b_up[:, None, :]], axis=1).astype(
        ml_dtypes.bfloat16
    )  # [E, HP, D]

    # expert table: row (e*P + p) = [wd cols for all dc | bd | -cs | pad
    # | W_up row p (HP rows, rest zero)]
    wdg = np.zeros((E, P, FW), dtype=ml_dtypes.bfloat16)
    wdg[:, :, : DC * H] = (
        wd_f.reshape(E, DC, P, H).transpose(0, 2, 1, 3).reshape(E, P, DC * H)
    )
    bdcs = np.stack([bd_f, -cs], axis=-1)  # [E, H, 2]
    wdg[:, :, 512:514] = np.concatenate([bdcs, bdcs], axis=1).astype(
        ml_dtypes.bfloat16
    )
    wdg[:, :HP, 640 : 640 + D] = wu_f
    wdg = wdg.reshape(E * P, FW)

    # gather ucode reads index i from slot [16 + i%16, i//16]; mirror into
    # partitions 0..15 too for the simulator's interpretation
    def _idx_table(n):
        t = np.zeros((P, n // 16), dtype=np.int16)
        t[:16, :] = np.arange(n, dtype=np.int16).reshape(n // 16, 16).T
        t[16:32, :] = t[:16, :]
        return t

    iota = np.ascontiguousarray(
        np.broadcast_to(np.arange(E, dtype=np.float32)[None], (P, E))
    )
    maskg = np.ascontiguousarray(
        np.tile(np.eye(M, dtype=ml_dtypes.bfloat16), (1, 3))
    )

    wr_hi = wr_f.astype(ml_dtypes.bfloat16)
    wr_lo = (wr_f - wr_hi.astype(np.float32)).astype(ml_dtypes.bfloat16)
    wr2 = np.concatenate([wr_hi, wr_lo], axis=1)  # [D, 2E]

    return {
        "ucbc": ucbc,
        "wdg": np.ascontiguousarray(wdg),
        "ident": np.eye(P, dtype=np.float32),
        "maskg": maskg,
        "iota": iota,
    }, wr2


def make_in_maps(inputs):
    params, wr2 = _fold_weights(inputs)
    x = np.asarray(inputs["x"], np.float32)
    x_bf = x.astype(ml_dtypes.bfloat16)
    wr_dc = np.ascontiguousarray(wr2.reshape(DC, P, 2 * E))  # [DC, P, 16]
    in_maps = []
    for i in range(NCORES):
        m = dict(params)
        xb = x_bf[i * BLOC : (i + 1) * BLOC]  # [BLOC, S, D]
        xp = np.zeros((BLOC, C * M, D), dtype=ml_dtypes.bfloat16)
        xp[:, :S] = xb
        xt = np.zeros((BLOC, P, DC, C, W), dtype=ml_dtypes.bfloat16)
        # [b, c, j, dc, p] -> [b, p, dc, c, j]
        xt[..., 0:M] = xp.reshape(BLOC, C, M, DC, P).transpose(0, 4, 3, 1, 2)
        xt[..., M : M + 2 * E] = wr_dc.transpose(1, 0, 2)[None, :, :, None, :]
        xt[..., P] = 1.0
        m["xt"] = np.ascontiguousarray(xt)
        in_maps.append(m)
    return in_maps


def get_nc():
    if "nc" not in _CACHE:
        _CACHE["nc"] = _build_kernel()
    return _CACHE["nc"]


def kernel(**inputs) -> np.ndarray:
    nc = get_nc()
    in_maps = make_in_maps(inputs)
    res = run_bass_kernel_spmd(nc, in_maps, core_ids=list(range(NCORES)))
    _CACHE["last_result"] = res
    out = np.concatenate(
        [
            np.asarray(res.results[i]["out"])
            .astype(np.float32)
            .transpose(0, 2, 1, 3)
            .reshape(BLOC, S, D)
            for i in range(NCORES)
        ],
        axis=0,
    )
    return out


if __name__ == "__main__":
    nc = get_nc()
    print("build + compile OK")
